# revision 1
# baseline (speedup 1.0000x reference)
"""Trainium2 8-core Bass kernel for the AnyAttention relation-gate module.

Strategy: shard the sequence axis (q) across 8 cores (256 rows each).
All LayerNorm weights are folded into the projection matrices host-side.
Per core: LN stats -> normalized x (bf16) -> PE-transpose -> projections
(transposed layout) -> per-(b,g) transposed scores -> exp (ScalarE, no max
subtraction; scores are O(1) by construction) -> per-head O = P @ [v|vw2|1]
matmuls giving attention outputs plus row sums l and weighted sums u for
free -> tiny 16-float AllReduce for the global relation-gate sum -> per-
partition weighted recombination -> gate matmul -> sigmoid -> output.
"""

from contextlib import ExitStack

import numpy as np
import ml_dtypes

BF16 = ml_dtypes.bfloat16
LAST_RESULT = None

NLOC = 256          # q rows per core
B = 2
N = 2048
D = 256
G = 8
C = 32
NCORES = 8
TOKK = B * N        # 4096 (b-major full tokens)
TOKQ = B * NLOC     # 512  (b-major local q tokens)
EPS = 1e-5
SCALE = float(C) ** -0.5
CREL = 1.0 / ((N - 1) * D)   # relation-gate normalizer


def _build(consts, repeat=1):
    """Build the Bass graph. consts: dict of host-computed scalar immediates."""
    import concourse.bacc as bacc
    import concourse.bass as bass
    import concourse.mybir as mybir
    import concourse.tile as tile

    f32 = mybir.dt.float32
    bf16 = mybir.dt.bfloat16
    AX = mybir.AxisListType.X
    OP = mybir.AluOpType
    ACT = mybir.ActivationFunctionType

    nc = bacc.Bacc(
        "TRN2", target_bir_lowering=False, debug=False, num_devices=NCORES
    )

    def din(name, shape, dt=f32):
        return nc.dram_tensor(name, list(shape), dt, kind="ExternalInput")

    q_in = din("q_sl", (TOKQ, D))
    k_in = din("k_in", (TOKK, D))
    v_in = din("v_in", (TOKK, D))
    wqT = din("wqT", (D, D), bf16)       # (d, e), ln_q_w folded
    wkT = din("wkT", (D, D), bf16)
    wvT = din("wvT", (D, D + 1), bf16)   # cols 0..255 proj, col 256 = w2e
    w1Tm = din("w1Tm", (D, D), bf16)     # (d, e): -CREL*mlp_w[e]*W1[e,d]
    biasq = din("biasq", (128, 2))       # (e%128, etile)
    biask = din("biask", (128, 2))
    svec_bc = din("svec_bc", (128, D))   # broadcast rows
    mlb_bc = din("mlb_bc", (128, D))
    bv_bc = din("bv_bc", (128, D))
    identb = din("identb", (128, 128), bf16)
    identf = din("identf", (128, 128))
    ones_col = din("ones_col", (128, 1))
    ones_row = din("ones_row", (1, 128))
    out_t = nc.dram_tensor("out", [TOKQ, D], f32, kind="ExternalOutput")

    b2s_eff = consts["b2s_eff"]          # bias_v@w2sum + b2sum

    MS = bass.MemorySpace

    with tile.TileContext(nc) as tc:
        with (
            tc.tile_pool(name="const", bufs=1) as cpool,
            tc.tile_pool(name="persist", bufs=1) as ppool,
            tc.tile_pool(name="work", bufs=1) as wpool,
            tc.tile_pool(name="stats", bufs=1) as stpool,
            tc.tile_pool(name="small", bufs=1) as spool,
            tc.tile_pool(name="fin", bufs=2) as fpool,
            tc.tile_pool(name="pt", bufs=3) as ptpool,
            tc.tile_pool(name="dram", bufs=1, space=MS.DRAM) as dpool,
        ):
            # ---- load small constants ----
            identb_sb = cpool.tile([128, 128], bf16, tag="identb")
            identf_sb = cpool.tile([128, 128], f32, tag="identf")
            wq_sb = cpool.tile([128, 2, D], bf16, tag="wq")
            wk_sb = cpool.tile([128, 2, D], bf16, tag="wk")
            wv_sb = cpool.tile([128, 2, D + 1], bf16, tag="wv")
            w1_sb = cpool.tile([128, 2, D], bf16, tag="w1")
            biasq_sb = cpool.tile([128, 2], f32, tag="biasq")
            biask_sb = cpool.tile([128, 2], f32, tag="biask")
            svec_sb = cpool.tile([128, D], f32, tag="svec")
            mlb_sb = cpool.tile([128, D], f32, tag="mlb")
            bv_sb = cpool.tile([128, D], f32, tag="bv")
            onec_sb = cpool.tile([128, 1], f32, tag="onec")
            oner_sb = cpool.tile([1, 128], f32, tag="oner")
            nc.sync.dma_start(identb_sb[:], identb[:])
            nc.sync.dma_start(identf_sb[:], identf[:])
            nc.sync.dma_start(wq_sb[:], wqT.ap().rearrange("(c p) e -> p c e", p=128))
            nc.sync.dma_start(wk_sb[:], wkT.ap().rearrange("(c p) e -> p c e", p=128))
            nc.sync.dma_start(wv_sb[:], wvT.ap().rearrange("(c p) e -> p c e", p=128))
            nc.sync.dma_start(w1_sb[:], w1Tm.ap().rearrange("(c p) e -> p c e", p=128))
            nc.sync.dma_start(biasq_sb[:], biasq[:])
            nc.sync.dma_start(biask_sb[:], biask[:])
            nc.sync.dma_start(svec_sb[:], svec_bc[:])
            nc.sync.dma_start(mlb_sb[:], mlb_bc[:])
            nc.sync.dma_start(bv_sb[:], bv_bc[:])
            nc.sync.dma_start(onec_sb[:], ones_col[:])
            nc.sync.dma_start(oner_sb[:], ones_row[:])

            # persistent activation tensors
            kpT = ppool.tile([128, 2, TOKK], bf16, tag="kpT")
            qpT = ppool.tile([128, 2, TOKQ], bf16, tag="qpT")
            vp = ppool.tile([128, TOKK // 128, D + 2], bf16, tag="vp")
            og = ppool.tile([128, 16, 2, D], bf16, tag="og")       # (bg, qh, d)
            l_st = ppool.tile([128, 2, 16], f32, tag="l_st")
            u_st = ppool.tile([128, 2, 16], f32, tag="u_st")
            G1 = ppool.tile([128, 2, 2, D], f32, tag="G1")         # (b, qh, d)
            G2 = ppool.tile([128, 2, 2, D], f32, tag="G2")

            def _pipeline():
                # ============ Phase A+B: LN + transpose + projections ============
                stackA = ExitStack()
                psA = stackA.enter_context(
                    tc.tile_pool(name="psA", bufs=2, space=MS.PSUM)
                )

                def ln_and_transpose(src_ap, ntiles):
                    """Load natural f32, LN per row, produce transposed bf16 (d, tok)."""
                    raw = wpool.tile([128, TOKK // 128, D], f32, tag="raw")
                    nc.sync.dma_start(
                        raw[:, 0:ntiles, :], src_ap.rearrange("(i p) d -> p i d", p=128)
                    )
                    stats6 = stpool.tile([128, TOKK // 128, 6], f32, tag="stats6")
                    mv = stpool.tile([128, TOKK // 128, 2], f32, tag="mv")
                    veps = stpool.tile([128, TOKK // 128], f32, tag="veps")
                    std = stpool.tile([128, TOKK // 128], f32, tag="std")
                    mean = stpool.tile([128, TOKK // 128], f32, tag="mean")
                    rstd = stpool.tile([128, TOKK // 128], f32, tag="rstd")
                    for i in range(ntiles):
                        nc.vector.bn_stats(stats6[:, i, :], raw[:, i, :])
                        nc.vector.bn_aggr(mv[:, i, :], stats6[:, i, :])
                    sl = slice(0, ntiles)
                    nc.vector.tensor_copy(mean[:, sl], mv[:, sl, 0])
                    nc.vector.tensor_scalar(
                        veps[:, sl], mv[:, sl, 1], EPS, None, op0=OP.add
                    )
                    # rstd = 1/sqrt(var+eps)
                    nc.scalar.activation(std[:, sl], veps[:, sl], ACT.Sqrt)
                    nc.vector.reciprocal(rstd[:, sl], std[:, sl])
                    xhat = wpool.tile([128, TOKK // 128, D], bf16, tag="xhat")
                    for i in range(ntiles):
                        nc.vector.tensor_scalar(
                            xhat[:, i, :],
                            raw[:, i, :],
                            mean[:, i : i + 1],
                            rstd[:, i : i + 1],
                            op0=OP.subtract,
                            op1=OP.mult,
                        )
                    xT = wpool.tile([128, 2, TOKK], bf16, tag="xT")
                    for i in range(ntiles):
                        for dc in range(2):
                            nc.sync.dma_start_transpose(
                                xT[:, dc, 128 * i : 128 * i + 128],
                                xhat[:, i, 128 * dc : 128 * dc + 128],
                            )
                    return xT

                # K
                xkT = ln_and_transpose(k_in.ap(), TOKK // 128)
                for et in range(2):
                    for t in range(TOKK // 512):
                        ps = psA.tile([128, 512], f32, tag="proj")
                        for dc in range(2):
                            nc.tensor.matmul(
                                ps[:],
                                wk_sb[:, dc, 128 * et : 128 * et + 128],
                                xkT[:, dc, 512 * t : 512 * t + 512],
                                start=(dc == 0),
                                stop=(dc == 1),
                            )
                        nc.vector.tensor_scalar(
                            kpT[:, et, 512 * t : 512 * t + 512],
                            ps[:],
                            biask_sb[:, et : et + 1],
                            None,
                            op0=OP.add,
                        )
                # V (uses xvT as stationary operand; natural-layout output)
                xvT = ln_and_transpose(v_in.ap(), TOKK // 128)
                for t in range(TOKK // 128):
                    ps = psA.tile([128, D + 1], f32, tag="projv")
                    for dc in range(2):
                        nc.tensor.matmul(
                            ps[:],
                            xvT[:, dc, 128 * t : 128 * t + 128],
                            wv_sb[:, dc, :],
                            start=(dc == 0),
                            stop=(dc == 1),
                        )
                    nc.vector.tensor_copy(vp[:, t, 0 : D + 1], ps[:])
                nc.vector.memset(vp[:, :, D + 1 : D + 2], 1.0)
                # Q
                xqT = ln_and_transpose(q_in.ap(), TOKQ // 128)
                for et in range(2):
                    ps = psA.tile([128, 512], f32, tag="proj")
                    for dc in range(2):
                        nc.tensor.matmul(
                            ps[:],
                            wq_sb[:, dc, 128 * et : 128 * et + 128],
                            xqT[:, dc, 0:TOKQ],
                            start=(dc == 0),
                            stop=(dc == 1),
                        )
                    nc.vector.tensor_scalar(
                        qpT[:, et, :],
                        ps[:],
                        biasq_sb[:, et : et + 1],
                        None,
                        op0=OP.add,
                    )

                # ============ Phase C: attention per (b, g) ============
                import os as _os
                _stage = _os.environ.get("KSTAGE", "full")
                if _stage == "AB":
                    dummy = spool.tile([128, D], f32, tag="dummy")
                    nc.vector.memset(dummy[:], 0.0)
                    for r in range(4):
                        nc.sync.dma_start(out_t[128 * r : 128 * r + 128, :], dummy[:])
                    stackA.close()
                    return
                stackA.close()
                stackC = ExitStack()
                psS = stackC.enter_context(
                    tc.tile_pool(name="psS", bufs=2, space=MS.PSUM)
                )
                psO = stackC.enter_context(
                    tc.tile_pool(name="psO", bufs=4, space=MS.PSUM)
                )
                for b in range(B):
                    for g in range(G):
                        bg = 8 * b + g
                        gp = 32 * (g % 4)
                        et = g // 4
                        PT = ptpool.tile([128, 16, NLOC], bf16, tag="PT")
                        for quarter in range(4):
                            ps_s = psS.tile([128, 4, NLOC], f32, tag="scores")
                            for kc4 in range(4):
                                kc = 4 * quarter + kc4
                                nc.tensor.matmul(
                                    ps_s[:, kc4, :],
                                    kpT[gp : gp + 32, et,
                                        2048 * b + 128 * kc : 2048 * b + 128 * kc + 128],
                                    qpT[gp : gp + 32, et, NLOC * b : NLOC * b + NLOC],
                                    tile_position=(gp, 0),
                                )
                            nc.scalar.activation(
                                PT[:, 4 * quarter : 4 * quarter + 4, :],
                                ps_s[:, :, :],
                                ACT.Exp,
                                scale=SCALE,
                            )
                        for qh in range(2):
                            ps_o = psO.tile([128, D + 2], f32, tag="O")
                            for kc in range(16):
                                nc.tensor.matmul(
                                    ps_o[:],
                                    PT[:, kc, 128 * qh : 128 * qh + 128],
                                    vp[:, 16 * b + kc, :],
                                    start=(kc == 0),
                                    stop=(kc == 15),
                                )
                            nc.vector.tensor_copy(og[:, bg, qh, :], ps_o[:, 0:D])
                            nc.vector.tensor_copy(
                                u_st[:, qh, bg : bg + 1], ps_o[:, D : D + 1]
                            )
                            nc.vector.tensor_copy(
                                l_st[:, qh, bg : bg + 1], ps_o[:, D + 1 : D + 2]
                            )

                # ============ Phase D: small stage + collective ============
                if _stage == "C":
                    dummy = spool.tile([128, D], f32, tag="dummy")
                    nc.vector.memset(dummy[:], 0.0)
                    for r in range(4):
                        nc.sync.dma_start(out_t[128 * r : 128 * r + 128, :], dummy[:])
                    stackC.close()
                    return
                stackC.close()
                stackD = ExitStack()
                psB = stackD.enter_context(
                    tc.tile_pool(name="psB", bufs=1, space=MS.PSUM)
                )
                recl = spool.tile([128, 2, 16], f32, tag="recl")
                tt = spool.tile([128, 2, 16], f32, tag="tt")
                od = spool.tile([128, 2, 16], f32, tag="od")
                w1s = spool.tile([128, 2, 16], f32, tag="w1s")
                w2s = spool.tile([128, 2, 16], f32, tag="w2s")
                s_st = spool.tile([128, 2, 2], f32, tag="s_st")
                tp_sb = spool.tile([16, 1], f32, tag="tp_sb")
                trow = spool.tile([1, 16], f32, tag="trow")

                nc.vector.reciprocal(recl[:], l_st[:])
                nc.vector.tensor_mul(tt[:], u_st[:], recl[:])
                ps_tp = psB.tile([16, 1], f32, tag="tpp")
                for qh in range(2):
                    nc.tensor.matmul(
                        ps_tp[:],
                        tt[:, qh, :],
                        onec_sb[:],
                        start=(qh == 0),
                        stop=(qh == 1),
                    )
                nc.vector.tensor_copy(tp_sb[:], ps_tp[:])
                ar_in = dpool.tile([16, 1], f32, tag="ar_in")
                ar_out = dpool.tile([16, 1], f32, tag="ar_out")
                nc.sync.dma_start(ar_in[:], tp_sb[:])
                nc.gpsimd.collective_compute(
                    "AllReduce",
                    OP.add,
                    ins=[ar_in.opt()],
                    outs=[ar_out.opt()],
                    replica_groups=[list(range(NCORES))],
                )
                nc.sync.dma_start(trow[:], ar_out[:].rearrange("a b -> b a"))
                ps_tbc = psB.tile([128, 16], f32, tag="tbc")
                nc.tensor.matmul(ps_tbc[:], oner_sb[:], trow[:])
                for qh in range(2):
                    nc.vector.tensor_sub(od[:, qh, :], ps_tbc[:], tt[:, qh, :])
                nc.vector.tensor_scalar(
                    od[:], od[:], (N - 1) * b2s_eff, None, op0=OP.add
                )
                nc.vector.tensor_mul(w1s[:], od[:], recl[:])
                nc.vector.tensor_scalar_mul(w2s[:], recl[:], 1.0 / G)
                for b in range(B):
                    for qh in range(2):
                        nc.vector.reduce_sum(
                            s_st[:, qh, b : b + 1], od[:, qh, 8 * b : 8 * b + 8], axis=AX
                        )

                # ============ Phase E: weighted recombination ============
                for b in range(B):
                    for qh in range(2):
                        for g in range(G):
                            bg = 8 * b + g
                            if g == 0:
                                nc.vector.tensor_scalar_mul(
                                    G1[:, b, qh, :], og[:, bg, qh, :],
                                    w1s[:, qh, bg : bg + 1],
                                )
                                nc.vector.tensor_scalar_mul(
                                    G2[:, b, qh, :], og[:, bg, qh, :],
                                    w2s[:, qh, bg : bg + 1],
                                )
                            else:
                                nc.vector.affine_then_add(
                                    G1[:, b, qh, :], og[:, bg, qh, :], G1[:, b, qh, :],
                                    scale=w1s[:, qh, bg : bg + 1], bias=0.0,
                                )
                                nc.vector.affine_then_add(
                                    G2[:, b, qh, :], og[:, bg, qh, :], G2[:, b, qh, :],
                                    scale=w2s[:, qh, bg : bg + 1], bias=0.0,
                                )

                # ============ Phase F: gate + output ============
                g1T = ppool.tile([128, 2, 4, 128], bf16, tag="g1T")
                for b in range(B):
                    for qh in range(2):
                        idx = 2 * b + qh
                        for dc in range(2):
                            ps = psB.tile([128, 128], f32, tag="g1tp")
                            nc.tensor.matmul(
                                ps[:],
                                G1[:, b, qh, 128 * dc : 128 * dc + 128],
                                identf_sb[:],
                                is_transpose=True,
                            )
                            nc.vector.tensor_copy(g1T[:, dc, idx, :], ps[:])
                for b in range(B):
                    for qh in range(2):
                        idx = 2 * b + qh
                        ps_a = psB.tile([128, D], f32, tag="A0")
                        for dc in range(2):
                            nc.tensor.matmul(
                                ps_a[:],
                                g1T[:, dc, idx, :],
                                w1_sb[:, dc, :],
                                start=(dc == 0),
                                stop=(dc == 1),
                            )
                        gi = fpool.tile([128, D], f32, tag="gi")
                        nc.vector.affine_then_add(
                            gi[:], svec_sb[:], ps_a[:],
                            scale=s_st[:, qh, b : b + 1], bias=0.0,
                        )
                        nc.vector.tensor_add(gi[:], gi[:], mlb_sb[:])
                        gate = fpool.tile([128, D], f32, tag="gate")
                        nc.scalar.activation(gate[:], gi[:], ACT.Sigmoid)
                        of = fpool.tile([128, D], f32, tag="of")
                        nc.vector.tensor_add(of[:], G2[:, b, qh, :], bv_sb[:])
                        nc.vector.tensor_mul(of[:], of[:], gate[:])
                        nc.sync.dma_start(
                            out_t[NLOC * b + 128 * qh : NLOC * b + 128 * qh + 128, :],
                            of[:],
                        )
                stackD.close()

            for _rep in range(repeat):
                _pipeline()

    return nc


def prepare(**inputs):
    q = np.asarray(inputs["q"], np.float32)
    k = np.asarray(inputs["k"], np.float32)
    v = np.asarray(inputs["v"], np.float32)
    ln_q_w = np.asarray(inputs["ln_q_w"], np.float64)
    ln_q_b = np.asarray(inputs["ln_q_b"], np.float64)
    ln_k_w = np.asarray(inputs["ln_k_w"], np.float64)
    ln_k_b = np.asarray(inputs["ln_k_b"], np.float64)
    ln_v_w = np.asarray(inputs["ln_v_w"], np.float64)
    ln_v_b = np.asarray(inputs["ln_v_b"], np.float64)
    Wq = np.asarray(inputs["Wq"], np.float64)
    Wk = np.asarray(inputs["Wk"], np.float64)
    Wv = np.asarray(inputs["Wv"], np.float64)
    W1 = np.asarray(inputs["W1"], np.float64)
    b1 = np.asarray(inputs["b1"], np.float64)
    W2 = np.asarray(inputs["W2"], np.float64)
    b2 = np.asarray(inputs["b2"], np.float64)
    mlp_w = np.asarray(inputs["mlp_w"], np.float64)
    mlp_b = np.asarray(inputs["mlp_b"], np.float64)

    # ---- host-side weight folding ----
    wqT = (Wq.T * ln_q_w[:, None]).astype(BF16)            # (d, e)
    wkT = (Wk.T * ln_k_w[:, None]).astype(BF16)
    wvT_eff = Wv.T * ln_v_w[:, None]                        # (d, e) float64
    biasq = (ln_q_b @ Wq.T).astype(np.float32)              # (e,)
    biask = (ln_k_b @ Wk.T).astype(np.float32)
    bias_v = ln_v_b @ Wv.T                                  # (e,) float64
    w2sum = W2.sum(axis=0)                                  # (d,)
    b2sum = float(b2.sum())
    w2e = wvT_eff @ w2sum                                   # (d,)
    wvT = np.concatenate([wvT_eff, w2e[:, None]], axis=1).astype(BF16)  # (d, 257)
    w1Tm_f = W1.T * (-CREL * mlp_w)[None, :]                # (d, e) float64
    w1Tm = w1Tm_f.astype(BF16)
    svec = (-CREL * mlp_w * b1 + bias_v @ w1Tm_f).astype(np.float32)
    b2s_eff = float(bias_v @ w2sum + b2sum)

    def bc(vec):
        return np.broadcast_to(
            np.asarray(vec, np.float32)[None, :], (128, D)
        ).copy()

    biasq_d = np.ascontiguousarray(biasq.reshape(2, 128).T)  # (128, 2)
    biask_d = np.ascontiguousarray(biask.reshape(2, 128).T)

    # ---- b-major activations ----
    k_bm = np.ascontiguousarray(k.transpose(1, 0, 2).reshape(TOKK, D))
    v_bm = np.ascontiguousarray(v.transpose(1, 0, 2).reshape(TOKK, D))
    q_bm = q.transpose(1, 0, 2)                             # (B, N, D)

    common = {
        "k_in": k_bm,
        "v_in": v_bm,
        "wqT": wqT,
        "wkT": wkT,
        "wvT": wvT,
        "w1Tm": w1Tm,
        "biasq": biasq_d,
        "biask": biask_d,
        "svec_bc": bc(svec),
        "mlb_bc": bc(mlp_b),
        "bv_bc": bc(bias_v),
        "identb": np.eye(128, dtype=BF16),
        "identf": np.eye(128, dtype=np.float32),
        "ones_col": np.ones((128, 1), np.float32),
        "ones_row": np.ones((1, 128), np.float32),
    }
    in_maps = []
    for i in range(NCORES):
        q_sl = np.ascontiguousarray(
            q_bm[:, i * NLOC : (i + 1) * NLOC, :].reshape(TOKQ, D)
        )
        in_maps.append({**common, "q_sl": q_sl})

    return in_maps, {"b2s_eff": b2s_eff}


def assemble(results):
    full = np.zeros((B, N, D), np.float32)
    for i in range(NCORES):
        o = np.asarray(results[i]["out"]).reshape(B, NLOC, D)
        full[:, i * NLOC : (i + 1) * NLOC, :] = o
    return np.ascontiguousarray(full.transpose(1, 0, 2))


def kernel(**inputs):
    from concourse import bass_utils

    in_maps, consts = prepare(**inputs)
    nc = _build(consts)
    nc.compile()
    res = bass_utils.run_bass_kernel_spmd(nc, in_maps, core_ids=list(range(NCORES)))
    global LAST_RESULT
    LAST_RESULT = res
    return assemble([res.results[i] for i in range(NCORES)])



# revision 13
# speedup vs baseline: 1.6383x; 1.6383x over previous
"""Trainium2 8-core Bass kernel for the AnyAttention relation-gate module.

Strategy: shard the sequence axis (q) across 8 cores (256 rows each).
All LayerNorm weights are folded into the projection matrices host-side.
Per core: LN stats -> normalized x (bf16) -> PE-transpose (TensorE, not
DMA-transpose) -> projections (transposed layout) -> per-(b,g) transposed
scores -> exp (ScalarE, no max subtraction; scores are O(1) by
construction) -> per-head O = P @ [v|vw2|1] matmuls giving attention
outputs plus row sums l and weighted sums u for free -> tiny 16-float
AllGather (+local sum) for the global relation-gate sum -> per-partition
weighted recombination -> gate matmul -> sigmoid -> output.
"""

from contextlib import ExitStack

import numpy as np
import ml_dtypes

BF16 = ml_dtypes.bfloat16
LAST_RESULT = None

NLOC = 256          # q rows per core
B = 2
N = 2048
D = 256
G = 8
C = 32
NCORES = 8
TOKK = B * N        # 4096 (b-major full tokens)
TOKQ = B * NLOC     # 512  (b-major local q tokens)
NTK = TOKK // 128   # 32 token tiles for K/V
NTQ = TOKQ // 128   # 4 token tiles for Q
EPS = 1e-5
SCALE = float(C) ** -0.5
CREL = 1.0 / ((N - 1) * D)   # relation-gate normalizer


def _build(consts, repeat=1):
    """Build the Bass graph. consts: dict of host-computed scalar immediates."""
    import concourse.bacc as bacc
    import concourse.bass as bass
    import concourse.mybir as mybir
    import concourse.tile as tile

    f32 = mybir.dt.float32
    bf16 = mybir.dt.bfloat16
    AX = mybir.AxisListType.X
    OP = mybir.AluOpType
    ACT = mybir.ActivationFunctionType

    nc = bacc.Bacc(
        "TRN2", target_bir_lowering=False, debug=False, num_devices=NCORES
    )

    def din(name, shape, dt=f32):
        return nc.dram_tensor(name, list(shape), dt, kind="ExternalInput")

    q_in = din("q_sl", (TOKQ, D))
    k_in = din("k_in", (TOKK, D))
    v_in = din("v_in", (TOKK, D))
    wqT = din("wqT", (D, D), bf16)       # (d, e), ln_q_w folded
    wkT = din("wkT", (D, D), bf16)
    wvT = din("wvT", (D, D + 1), bf16)   # cols 0..255 proj, col 256 = w2e
    w1Tm = din("w1Tm", (D, D), bf16)     # (d, e): -CREL*mlp_w[e]*W1[e,d]
    biasq = din("biasq", (128, 2))       # (e%128, etile)
    biask = din("biask", (128, 2))
    svec_bc = din("svec_bc", (128, D))   # broadcast rows
    mlb_bc = din("mlb_bc", (128, D))
    bv_bc = din("bv_bc", (128, D))
    identb = din("identb", (128, 128), bf16)
    identf = din("identf", (128, 128))
    ones_col = din("ones_col", (128, 1))
    ones_row = din("ones_row", (1, 128))
    out_t = nc.dram_tensor("out", [TOKQ, D], f32, kind="ExternalOutput")

    b2s_eff = consts["b2s_eff"]          # bias_v@w2sum + b2sum

    MS = bass.MemorySpace

    with tile.TileContext(nc) as tc:
        with (
            tc.tile_pool(name="const", bufs=1) as cpool,
            tc.tile_pool(name="persist", bufs=1) as ppool,
            tc.tile_pool(name="lnw", bufs=12) as lnpool,
            tc.tile_pool(name="stats", bufs=1) as stpool,
            tc.tile_pool(name="small", bufs=1) as spool,
            tc.tile_pool(name="fin", bufs=2) as fpool,
            tc.tile_pool(name="pt", bufs=3) as ptpool,
            tc.tile_pool(name="dram", bufs=1, space=MS.DRAM) as dpool,
        ):
            # ---- load small constants ----
            identb_sb = cpool.tile([128, 128], bf16, tag="identb")
            identf_sb = cpool.tile([128, 128], f32, tag="identf")
            wq_sb = cpool.tile([128, 2, D], bf16, tag="wq")
            wk_sb = cpool.tile([128, 2, D], bf16, tag="wk")
            wv_sb = cpool.tile([128, 2, D + 1], bf16, tag="wv")
            w1_sb = cpool.tile([128, 2, D], bf16, tag="w1")
            biasq_sb = cpool.tile([128, 2], f32, tag="biasq")
            biask_sb = cpool.tile([128, 2], f32, tag="biask")
            svec_sb = cpool.tile([128, D], f32, tag="svec")
            mlb_sb = cpool.tile([128, D], f32, tag="mlb")
            bv_sb = cpool.tile([128, D], f32, tag="bv")
            onec_sb = cpool.tile([128, 1], f32, tag="onec")
            oner_sb = cpool.tile([1, 128], f32, tag="oner")
            nc.sync.dma_start(identb_sb[:], identb[:])
            nc.sync.dma_start(identf_sb[:], identf[:])
            nc.sync.dma_start(wq_sb[:], wqT.ap().rearrange("(c p) e -> p c e", p=128))
            nc.sync.dma_start(wk_sb[:], wkT.ap().rearrange("(c p) e -> p c e", p=128))
            nc.sync.dma_start(wv_sb[:], wvT.ap().rearrange("(c p) e -> p c e", p=128))
            nc.sync.dma_start(w1_sb[:], w1Tm.ap().rearrange("(c p) e -> p c e", p=128))
            nc.sync.dma_start(biasq_sb[:], biasq[:])
            nc.sync.dma_start(biask_sb[:], biask[:])
            nc.sync.dma_start(svec_sb[:], svec_bc[:])
            nc.sync.dma_start(mlb_sb[:], mlb_bc[:])
            nc.sync.dma_start(bv_sb[:], bv_bc[:])
            nc.sync.dma_start(onec_sb[:], ones_col[:])
            nc.sync.dma_start(oner_sb[:], ones_row[:])

            # persistent activation tensors
            kpT = ppool.tile([128, 2, TOKK], bf16, tag="kpT")
            qpT = ppool.tile([128, 2, TOKQ], bf16, tag="qpT")
            vp = ppool.tile([128, NTK, D + 2], bf16, tag="vp")
            og = ppool.tile([128, 16, 2, D], bf16, tag="og")       # (bg, qh, d)
            ul = ppool.tile([128, 16, 2, 2], f32, tag="ul")        # (bg, qh, {u,l})
            G1 = ppool.tile([128, 2, 2, D], f32, tag="G1")         # (b, qh, d)
            G2 = ppool.tile([128, 2, 2, D], f32, tag="G2")

            def _pipeline():
                # ============ Phase A: LN + PE-transpose + projections ======
                stackA = ExitStack()
                psT = stackA.enter_context(
                    tc.tile_pool(name="psT", bufs=2, space=MS.PSUM)
                )
                psA = stackA.enter_context(
                    tc.tile_pool(name="psA", bufs=2, space=MS.PSUM)
                )
                psV = stackA.enter_context(
                    tc.tile_pool(name="psV", bufs=2, space=MS.PSUM)
                )

                def ln_to_xT(src_t, ntiles, tg):
                    """Load natural f32, LN per row (stats on DVE, apply on
                    ScalarE via Identity(scale*x+bias)), produce transposed
                    bf16 xT [128, 2, ntiles*128] via TensorE transposes."""
                    st6 = stpool.tile([128, ntiles, 6], f32, tag=f"st6{tg}")
                    mv = stpool.tile([128, ntiles, 2], f32, tag=f"mv{tg}")
                    veps = stpool.tile([128, ntiles], f32, tag=f"veps{tg}")
                    std = stpool.tile([128, ntiles], f32, tag=f"std{tg}")
                    rstd = stpool.tile([128, ntiles], f32, tag=f"rstd{tg}")
                    nmr = stpool.tile([128, ntiles], f32, tag=f"nmr{tg}")
                    xh = stpool.tile([128, ntiles, D], bf16, tag=f"xh{tg}")
                    xT = stpool.tile([128, 2, ntiles * 128], bf16, tag=f"xT{tg}")
                    nhalf = 2 if ntiles >= 8 else 1
                    tph = ntiles // nhalf          # tiles per half
                    for h in range(nhalf):
                        raws = []
                        for jj in range(tph // 2):
                            j = h * (tph // 2) + jj
                            raw = lnpool.tile([128, 2, D], f32, tag="raw")
                            raws.append(raw)
                            nc.sync.dma_start(
                                raw[:],
                                src_t[256 * j : 256 * (j + 1), :].rearrange(
                                    "(t p) d -> p t d", p=128
                                ),
                            )
                            for t2 in range(2):
                                nc.vector.bn_stats(
                                    st6[:, 2 * j + t2, :], raw[:, t2, :]
                                )
                                nc.vector.bn_aggr(
                                    mv[:, 2 * j + t2, :], st6[:, 2 * j + t2, :]
                                )
                        sl = slice(h * tph, (h + 1) * tph)
                        nc.vector.tensor_scalar(
                            veps[:, sl], mv[:, sl, 1], EPS, None, op0=OP.add
                        )
                        nc.scalar.activation(std[:, sl], veps[:, sl], ACT.Sqrt)
                        nc.vector.reciprocal(rstd[:, sl], std[:, sl])
                        nc.vector.scalar_tensor_tensor(
                            nmr[:, sl], mv[:, sl, 0], -1.0, rstd[:, sl],
                            op0=OP.mult, op1=OP.mult,
                        )
                        # xh = rstd*x - mean*rstd on ScalarE (Identity is in
                        # every ACT table set: no table switches)
                        for i in range(h * tph, (h + 1) * tph):
                            jj, t2 = divmod(i - h * tph, 2)
                            nc.scalar.activation(
                                xh[:, i, :], raws[jj][:, t2, :], ACT.Identity,
                                bias=nmr[:, i : i + 1], scale=rstd[:, i : i + 1],
                            )
                    for j in range(ntiles // 4 if ntiles >= 4 else 1):
                        ps = psT.tile([128, 4, 2, 128], bf16, tag="tr")
                        for t4 in range(4):
                            i = 4 * j + t4
                            for dc in range(2):
                                nc.tensor.transpose(
                                    ps[:, t4, dc, :],
                                    xh[:, i, 128 * dc : 128 * (dc + 1)],
                                    identb_sb[:],
                                )
                        # one cast per 4 token-tiles: (t4, dc, col) -> (dc, t4, col)
                        src = ps[:].rearrange("p t d c -> p d t c")
                        dst = xT[:, :, 512 * j : 512 * (j + 1)].rearrange(
                            "p d (t c) -> p d t c", c=128
                        )
                        nc.vector.tensor_copy(dst, src)
                    return xT

                # Q first (small; unblocks scores early), then K, then V.
                xqT = ln_to_xT(q_in.ap(), NTQ, "q")
                for et in range(2):
                    ps = psA.tile([128, 512], f32, tag="projk")
                    for dc in range(2):
                        nc.tensor.matmul(
                            ps[:],
                            wq_sb[:, dc, 128 * et : 128 * (et + 1)],
                            xqT[:, dc, 0:TOKQ],
                            start=(dc == 0),
                            stop=(dc == 1),
                        )
                    nc.vector.tensor_scalar(
                        qpT[:, et, :], ps[:], biasq_sb[:, et : et + 1], None,
                        op0=OP.add,
                    )
                xkT = ln_to_xT(k_in.ap(), NTK, "k")
                for et in range(2):
                    for t in range(TOKK // 1024):
                        ps = psA.tile([128, 1024], f32, tag="projk")
                        for half in range(2):
                            hsl = slice(512 * half, 512 * (half + 1))
                            for dc in range(2):
                                nc.tensor.matmul(
                                    ps[:, hsl],
                                    wk_sb[:, dc, 128 * et : 128 * (et + 1)],
                                    xkT[:, dc, 1024 * t + 512 * half :
                                        1024 * t + 512 * (half + 1)],
                                    start=(dc == 0),
                                    stop=(dc == 1),
                                )
                        nc.vector.tensor_scalar(
                            kpT[:, et, 1024 * t : 1024 * (t + 1)],
                            ps[:], biask_sb[:, et : et + 1], None,
                            op0=OP.add,
                        )
                xvT = ln_to_xT(v_in.ap(), NTK, "v")
                for t in range(NTK):
                    ps = psV.tile([128, D + 1], f32, tag="projv")
                    for dc in range(2):
                        nc.tensor.matmul(
                            ps[:],
                            xvT[:, dc, 128 * t : 128 * (t + 1)],
                            wv_sb[:, dc, :],
                            start=(dc == 0),
                            stop=(dc == 1),
                        )
                    nc.vector.tensor_copy(vp[:, t, 0 : D + 1], ps[:])
                nc.vector.memset(vp[:, :, D + 1 : D + 2], 1.0)

                # ============ Phase C: attention per (b, g) ============
                import os as _os
                _stage = _os.environ.get("KSTAGE", "full")
                if _stage == "AB":
                    dummy = spool.tile([128, D], f32, tag="dummy")
                    nc.vector.memset(dummy[:], 0.0)
                    for r in range(4):
                        nc.sync.dma_start(out_t[128 * r : 128 * r + 128, :], dummy[:])
                    stackA.close()
                    return
                stackA.close()
                stackC = ExitStack()
                psS = stackC.enter_context(
                    tc.tile_pool(name="psS", bufs=2, space=MS.PSUM)
                )
                psO = stackC.enter_context(
                    tc.tile_pool(name="psO", bufs=4, space=MS.PSUM)
                )
                for b in range(B):
                    for g in range(G):
                        bg = 8 * b + g
                        gp = 32 * (g % 4)
                        et = g // 4
                        PT = ptpool.tile([128, 16, NLOC], bf16, tag="PT")
                        for quarter in range(4):
                            ps_s = psS.tile([128, 4, NLOC], f32, tag="scores")
                            for kc4 in range(4):
                                kc = 4 * quarter + kc4
                                nc.tensor.matmul(
                                    ps_s[:, kc4, :],
                                    kpT[gp : gp + 32, et,
                                        2048 * b + 128 * kc : 2048 * b + 128 * kc + 128],
                                    qpT[gp : gp + 32, et, NLOC * b : NLOC * b + NLOC],
                                    tile_position=(gp, 0),
                                )
                            nc.scalar.activation(
                                PT[:, 4 * quarter : 4 * quarter + 4, :],
                                ps_s[:, :, :],
                                ACT.Exp,
                                scale=SCALE,
                            )
                        for qh in range(2):
                            ps_o = psO.tile([128, D + 2], f32, tag="O")
                            for kc in range(16):
                                nc.tensor.matmul(
                                    ps_o[:],
                                    PT[:, kc, 128 * qh : 128 * qh + 128],
                                    vp[:, 16 * b + kc, :],
                                    start=(kc == 0),
                                    stop=(kc == 15),
                                )
                            nc.vector.tensor_copy(og[:, bg, qh, :], ps_o[:, 0:D])
                            nc.vector.tensor_copy(
                                ul[:, bg, qh, :], ps_o[:, D : D + 2]
                            )

                # ============ Phase D: small stage + collective ============
                if _stage == "C":
                    dummy = spool.tile([128, D], f32, tag="dummy")
                    nc.vector.memset(dummy[:], 0.0)
                    for r in range(4):
                        nc.sync.dma_start(out_t[128 * r : 128 * r + 128, :], dummy[:])
                    stackC.close()
                    return
                stackC.close()
                stackD = ExitStack()
                psB = stackD.enter_context(
                    tc.tile_pool(name="psB", bufs=2, space=MS.PSUM)
                )
                recl = spool.tile([128, 16, 2], f32, tag="recl")
                tt = spool.tile([128, 16, 2], f32, tag="tt")
                od = spool.tile([128, 16, 2], f32, tag="od")
                w1s = spool.tile([128, 16, 2], f32, tag="w1s")
                w2s = spool.tile([128, 16, 2], f32, tag="w2s")
                s_st = spool.tile([128, 2, 2], f32, tag="s_st")
                trow = spool.tile([1, 16], f32, tag="trow")
                trow8 = spool.tile([1, 128], f32, tag="trow8")
                t64 = spool.tile([1, 64], f32, tag="t64")
                t32 = spool.tile([1, 32], f32, tag="t32")

                nc.vector.reciprocal(recl[:], ul[:, :, :, 1])
                nc.vector.tensor_mul(tt[:], ul[:, :, :, 0], recl[:])
                ps_tp = psB.tile([16, 1], f32, tag="tpp")
                for qh in range(2):
                    nc.tensor.matmul(
                        ps_tp[:],
                        tt[:, :, qh],
                        onec_sb[:],
                        start=(qh == 0),
                        stop=(qh == 1),
                    )
                tp_sb = spool.tile([16, 1], f32, tag="tp_sb")
                nc.vector.tensor_copy(tp_sb[:], ps_tp[:])
                ag_in = dpool.tile([16, 1], f32, tag="ag_in")
                ag_out = dpool.tile([128, 1], f32, tag="ag_out")
                nc.sync.dma_start(ag_in[:], tp_sb[:])
                nc.gpsimd.collective_compute(
                    "AllGather",
                    OP.bypass,
                    ins=[ag_in.opt()],
                    outs=[ag_out.opt()],
                    replica_groups=[list(range(NCORES))],
                )
                nc.sync.dma_start(trow8[:], ag_out[:].rearrange("a b -> b a"))
                # Collective-independent DVE work issued BEFORE any
                # collective-gated DVE op (DVE executes strictly in order):
                # w2s + the G2 recombination overlap the collective wait.
                nc.vector.tensor_scalar_mul(w2s[:], recl[:], 1.0 / G)
                for b in range(B):
                    for qh in range(2):
                        for g in range(G):
                            bg = 8 * b + g
                            if g == 0:
                                nc.vector.tensor_scalar_mul(
                                    G2[:, b, qh, :], og[:, bg, qh, :],
                                    w2s[:, bg, qh : qh + 1],
                                )
                            else:
                                nc.vector.affine_then_add(
                                    G2[:, b, qh, :], og[:, bg, qh, :], G2[:, b, qh, :],
                                    scale=w2s[:, bg, qh : qh + 1], bias=0.0,
                                )
                        # fold the +bias_v term in while the collective runs
                        nc.vector.tensor_add(
                            G2[:, b, qh, :], G2[:, b, qh, :], bv_sb[:]
                        )
                nc.vector.tensor_add(t64[:], trow8[:, 0:64], trow8[:, 64:128])
                nc.vector.tensor_add(t32[:], t64[:, 0:32], t64[:, 32:64])
                nc.vector.tensor_add(trow[:], t32[:, 0:16], t32[:, 16:32])
                ps_tbc = psB.tile([128, 16], f32, tag="tbc")
                nc.tensor.matmul(ps_tbc[:], oner_sb[:], trow[:])
                for qh in range(2):
                    nc.vector.tensor_sub(od[:, :, qh], ps_tbc[:], tt[:, :, qh])
                nc.vector.tensor_scalar(
                    od[:], od[:], (N - 1) * b2s_eff, None, op0=OP.add
                )
                nc.vector.tensor_mul(w1s[:], od[:], recl[:])
                for b in range(B):
                    for qh in range(2):
                        nc.vector.reduce_sum(
                            s_st[:, qh, b : b + 1], od[:, 8 * b : 8 * b + 8, qh],
                            axis=AX,
                        )

                # ============ Phase E: G1 recombination (collective-gated) ==
                for b in range(B):
                    for qh in range(2):
                        for g in range(G):
                            bg = 8 * b + g
                            if g == 0:
                                nc.vector.tensor_scalar_mul(
                                    G1[:, b, qh, :], og[:, bg, qh, :],
                                    w1s[:, bg, qh : qh + 1],
                                )
                            else:
                                nc.vector.affine_then_add(
                                    G1[:, b, qh, :], og[:, bg, qh, :], G1[:, b, qh, :],
                                    scale=w1s[:, bg, qh : qh + 1], bias=0.0,
                                )

                # ============ Phase F: gate + output ============
                g1T = ppool.tile([128, 2, 4, 128], bf16, tag="g1T")
                for b in range(B):
                    for qh in range(2):
                        idx = 2 * b + qh
                        for dc in range(2):
                            ps = psB.tile([128, 128], f32, tag="g1tp")
                            nc.tensor.matmul(
                                ps[:],
                                G1[:, b, qh, 128 * dc : 128 * dc + 128],
                                identf_sb[:],
                                is_transpose=True,
                            )
                            nc.vector.tensor_copy(g1T[:, dc, idx, :], ps[:])
                for b in range(B):
                    for qh in range(2):
                        idx = 2 * b + qh
                        ps_a = psB.tile([128, D], f32, tag="A0")
                        for dc in range(2):
                            nc.tensor.matmul(
                                ps_a[:],
                                g1T[:, dc, idx, :],
                                w1_sb[:, dc, :],
                                start=(dc == 0),
                                stop=(dc == 1),
                            )
                        gi = fpool.tile([128, D], f32, tag="gi")
                        nc.vector.affine_then_add(
                            gi[:], svec_sb[:], ps_a[:],
                            scale=s_st[:, qh, b : b + 1], bias=0.0,
                        )
                        nc.vector.tensor_add(gi[:], gi[:], mlb_sb[:])
                        gate = fpool.tile([128, D], f32, tag="gate")
                        nc.scalar.activation(gate[:], gi[:], ACT.Sigmoid)
                        of = fpool.tile([128, D], f32, tag="of")
                        nc.vector.tensor_mul(of[:], G2[:, b, qh, :], gate[:])
                        nc.sync.dma_start(
                            out_t[NLOC * b + 128 * qh : NLOC * b + 128 * qh + 128, :],
                            of[:],
                        )
                stackD.close()

            for _rep in range(repeat):
                _pipeline()

    return nc


def prepare(**inputs):
    q = np.asarray(inputs["q"], np.float32)
    k = np.asarray(inputs["k"], np.float32)
    v = np.asarray(inputs["v"], np.float32)
    ln_q_w = np.asarray(inputs["ln_q_w"], np.float64)
    ln_q_b = np.asarray(inputs["ln_q_b"], np.float64)
    ln_k_w = np.asarray(inputs["ln_k_w"], np.float64)
    ln_k_b = np.asarray(inputs["ln_k_b"], np.float64)
    ln_v_w = np.asarray(inputs["ln_v_w"], np.float64)
    ln_v_b = np.asarray(inputs["ln_v_b"], np.float64)
    Wq = np.asarray(inputs["Wq"], np.float64)
    Wk = np.asarray(inputs["Wk"], np.float64)
    Wv = np.asarray(inputs["Wv"], np.float64)
    W1 = np.asarray(inputs["W1"], np.float64)
    b1 = np.asarray(inputs["b1"], np.float64)
    W2 = np.asarray(inputs["W2"], np.float64)
    b2 = np.asarray(inputs["b2"], np.float64)
    mlp_w = np.asarray(inputs["mlp_w"], np.float64)
    mlp_b = np.asarray(inputs["mlp_b"], np.float64)

    # ---- host-side weight folding ----
    wqT = (Wq.T * ln_q_w[:, None]).astype(BF16)            # (d, e)
    wkT = (Wk.T * ln_k_w[:, None]).astype(BF16)
    wvT_eff = Wv.T * ln_v_w[:, None]                        # (d, e) float64
    biasq = (ln_q_b @ Wq.T).astype(np.float32)              # (e,)
    biask = (ln_k_b @ Wk.T).astype(np.float32)
    bias_v = ln_v_b @ Wv.T                                  # (e,) float64
    w2sum = W2.sum(axis=0)                                  # (d,)
    b2sum = float(b2.sum())
    w2e = wvT_eff @ w2sum                                   # (d,)
    wvT = np.concatenate([wvT_eff, w2e[:, None]], axis=1).astype(BF16)  # (d, 257)
    w1Tm_f = W1.T * (-CREL * mlp_w)[None, :]                # (d, e) float64
    w1Tm = w1Tm_f.astype(BF16)
    svec = (-CREL * mlp_w * b1 + bias_v @ w1Tm_f).astype(np.float32)
    b2s_eff = float(bias_v @ w2sum + b2sum)

    def bc(vec):
        return np.broadcast_to(
            np.asarray(vec, np.float32)[None, :], (128, D)
        ).copy()

    biasq_d = np.ascontiguousarray(biasq.reshape(2, 128).T)  # (128, 2)
    biask_d = np.ascontiguousarray(biask.reshape(2, 128).T)

    # ---- b-major activations ----
    k_bm = np.ascontiguousarray(k.transpose(1, 0, 2).reshape(TOKK, D))
    v_bm = np.ascontiguousarray(v.transpose(1, 0, 2).reshape(TOKK, D))
    q_bm = q.transpose(1, 0, 2)                             # (B, N, D)

    common = {
        "k_in": k_bm,
        "v_in": v_bm,
        "wqT": wqT,
        "wkT": wkT,
        "wvT": wvT,
        "w1Tm": w1Tm,
        "biasq": biasq_d,
        "biask": biask_d,
        "svec_bc": bc(svec),
        "mlb_bc": bc(mlp_b),
        "bv_bc": bc(bias_v),
        "identb": np.eye(128, dtype=BF16),
        "identf": np.eye(128, dtype=np.float32),
        "ones_col": np.ones((128, 1), np.float32),
        "ones_row": np.ones((1, 128), np.float32),
    }
    in_maps = []
    for i in range(NCORES):
        q_sl = np.ascontiguousarray(
            q_bm[:, i * NLOC : (i + 1) * NLOC, :].reshape(TOKQ, D)
        )
        in_maps.append({**common, "q_sl": q_sl})

    return in_maps, {"b2s_eff": b2s_eff}


def assemble(results):
    full = np.zeros((B, N, D), np.float32)
    for i in range(NCORES):
        o = np.asarray(results[i]["out"]).reshape(B, NLOC, D)
        full[:, i * NLOC : (i + 1) * NLOC, :] = o
    return np.ascontiguousarray(full.transpose(1, 0, 2))


def kernel(**inputs):
    from concourse import bass_utils

    in_maps, consts = prepare(**inputs)
    nc = _build(consts)
    nc.compile()
    res = bass_utils.run_bass_kernel_spmd(nc, in_maps, core_ids=list(range(NCORES)))
    global LAST_RESULT
    LAST_RESULT = res
    return assemble([res.results[i] for i in range(NCORES)])


# revision 21
# speedup vs baseline: 1.8629x; 1.1371x over previous
"""Trainium2 8-core Bass kernel for the AnyAttention relation-gate module.

Strategy: shard the sequence axis (q) across 8 cores (256 rows each).
All LayerNorm weights are folded into the projection matrices host-side.
Per core: LN stats -> normalized x (bf16) -> PE-transpose (TensorE) ->
projections (transposed layout) -> per-(b,g) transposed scores -> exp
(ScalarE, no max subtraction) -> per-head O = P @ [v|vw2|1] matmuls giving
attention outputs plus row sums l and weighted sums u for free -> tiny
16-float AllGather (+local sum) for the global relation-gate term ->
per-partition weighted recombination -> gate matmul -> sigmoid -> output.

Scheduling: K and Q are normalized/projected first (LN-apply on ScalarE
via Identity, which lives in every ACT table set); V's LN-apply runs on
VectorE (centering only; rstd is folded into the projection cast as a
per-partition multiply) so ScalarE is free to run the 64 Exp ops as soon
as the first scores land. PSUM banks: scores(4) + V transposes(2) +
V projections(2) coexist; PV outputs reuse the projection slots.
"""

from contextlib import ExitStack

import numpy as np
import ml_dtypes

BF16 = ml_dtypes.bfloat16
LAST_RESULT = None

NLOC = 256          # q rows per core
B = 2
N = 2048
D = 256
G = 8
C = 32
NCORES = 8
TOKK = B * N        # 4096 (b-major full tokens)
TOKQ = B * NLOC     # 512  (b-major local q tokens)
NTK = TOKK // 128   # 32 token tiles for K/V
NTQ = TOKQ // 128   # 4 token tiles for Q
EPS = 1e-5
SCALE = float(C) ** -0.5
CREL = 1.0 / ((N - 1) * D)   # relation-gate normalizer

# const blob column offsets (bf16 blob)
CB_IDENT = 0
CB_WQ = 128              # 2*256
CB_WK = CB_WQ + 512      # 2*256
CB_WV = CB_WK + 512      # 2*257
CB_W1 = CB_WV + 514      # 2*256
CB_COLS = CB_W1 + 512
# f32 blob
CF_IDENT = 0
CF_SVEC = 128
CF_MLB = CF_SVEC + D
CF_BV = CF_MLB + D
CF_BIASQ = CF_BV + D     # 2 cols
CF_BIASK = CF_BIASQ + 2  # 2 cols
CF_ONEC = CF_BIASK + 2   # 1 col
CF_COLS = CF_ONEC + 1


def _build(consts, repeat=1):
    """Build the Bass graph. consts: dict of host-computed scalar immediates."""
    import concourse.bacc as bacc
    import concourse.bass as bass
    import concourse.mybir as mybir
    import concourse.tile as tile

    f32 = mybir.dt.float32
    bf16 = mybir.dt.bfloat16
    AX = mybir.AxisListType.X
    OP = mybir.AluOpType
    ACT = mybir.ActivationFunctionType

    nc = bacc.Bacc(
        "TRN2", target_bir_lowering=False, debug=False, num_devices=NCORES
    )

    def din(name, shape, dt=f32):
        return nc.dram_tensor(name, list(shape), dt, kind="ExternalInput")

    q_in = din("q_sl", (TOKQ, D))
    k_in = din("k_in", (TOKK, D))
    v_in = din("v_in", (TOKK, D))
    cb_in = din("cblob", (128, CB_COLS), bf16)
    cf_in = din("cfblob", (128, CF_COLS))
    oner_in = din("ones_row", (1, 128))
    out_t = nc.dram_tensor("out", [TOKQ, D], f32, kind="ExternalOutput")

    b2s_eff = consts["b2s_eff"]          # bias_v@w2sum + b2sum

    MS = bass.MemorySpace

    with tile.TileContext(nc) as tc:
        with (
            tc.tile_pool(name="const", bufs=1) as cpool,
            tc.tile_pool(name="persist", bufs=1) as ppool,
            tc.tile_pool(name="lnw", bufs=12) as lnpool,
            tc.tile_pool(name="xh", bufs=6) as xhpool,
            tc.tile_pool(name="stats", bufs=1) as stpool,
            tc.tile_pool(name="small", bufs=1) as spool,
            tc.tile_pool(name="fin", bufs=2) as fpool,
            tc.tile_pool(name="pt", bufs=8) as ptpool,
            tc.tile_pool(name="dram", bufs=1, space=MS.DRAM) as dpool,
        ):
            # ---- batched constant loads (3 DMAs) ----
            cb = cpool.tile([128, CB_COLS], bf16, tag="cb")
            cf = cpool.tile([128, CF_COLS], f32, tag="cf")
            oner_sb = cpool.tile([1, 128], f32, tag="oner")
            nc.sync.dma_start(cb[:], cb_in[:])
            nc.sync.dma_start(cf[:], cf_in[:])
            nc.sync.dma_start(oner_sb[:], oner_in[:])

            identb_sb = cb[:, CB_IDENT : CB_IDENT + 128]
            identf_sb = cf[:, CF_IDENT : CF_IDENT + 128]
            svec_sb = cf[:, CF_SVEC : CF_SVEC + D]
            mlb_sb = cf[:, CF_MLB : CF_MLB + D]
            bv_sb = cf[:, CF_BV : CF_BV + D]
            onec_sb = cf[:, CF_ONEC : CF_ONEC + 1]

            def wq_s(dc, lo, n):
                return cb[:, CB_WQ + 256 * dc + lo : CB_WQ + 256 * dc + lo + n]

            def wk_s(dc, lo, n):
                return cb[:, CB_WK + 256 * dc + lo : CB_WK + 256 * dc + lo + n]

            def wv_s(dc):
                return cb[:, CB_WV + 257 * dc : CB_WV + 257 * (dc + 1)]

            def w1_s(dc):
                return cb[:, CB_W1 + 256 * dc : CB_W1 + 256 * (dc + 1)]

            # persistent activation tensors
            kpT = ppool.tile([128, 2, TOKK], bf16, tag="kpT")
            qpT = ppool.tile([128, 2, TOKQ], bf16, tag="qpT")
            vp = ppool.tile([128, NTK, D + 2], bf16, tag="vp")
            og = ppool.tile([128, 16, 2, D], bf16, tag="og")       # (bg, qh, d)
            ul = ppool.tile([128, 16, 2, 2], f32, tag="ul")        # (bg, qh, {u,l})
            G1 = ppool.tile([128, 2, 2, D], f32, tag="G1")         # (b, qh, d)
            G2 = ppool.tile([128, 2, 2, D], f32, tag="G2")

            def _pipeline():
                stackA = ExitStack()
                psT = stackA.enter_context(
                    tc.tile_pool(name="psT", bufs=2, space=MS.PSUM)
                )
                psA = stackA.enter_context(
                    tc.tile_pool(name="psA", bufs=2, space=MS.PSUM)
                )
                psS = stackA.enter_context(
                    tc.tile_pool(name="psS", bufs=2, space=MS.PSUM)
                )

                def ln_stats(src_t, ntiles, tg):
                    """Per-tile loads + BN stats; batched rstd per half."""
                    st6 = stpool.tile([128, ntiles, 6], f32, tag=f"st6{tg}")
                    mv = stpool.tile([128, ntiles, 2], f32, tag=f"mv{tg}")
                    veps = stpool.tile([128, ntiles], f32, tag=f"veps{tg}")
                    std = stpool.tile([128, ntiles], f32, tag=f"std{tg}")
                    rstd = stpool.tile([128, ntiles], f32, tag=f"rstd{tg}")
                    nmr = stpool.tile([128, ntiles], f32, tag=f"nmr{tg}")
                    nhalf = 2 if ntiles >= 8 else 1
                    tph = ntiles // nhalf
                    raws = []
                    for h in range(nhalf):
                        for jj in range(tph // 2):
                            j = h * (tph // 2) + jj
                            raw = lnpool.tile([128, 2, D], f32, tag="raw")
                            raws.append(raw)
                            nc.sync.dma_start(
                                raw[:],
                                src_t[256 * j : 256 * (j + 1), :].rearrange(
                                    "(t p) d -> p t d", p=128
                                ),
                            )
                            for t2 in range(2):
                                nc.vector.bn_stats(
                                    st6[:, 2 * j + t2, :], raw[:, t2, :]
                                )
                                nc.vector.bn_aggr(
                                    mv[:, 2 * j + t2, :], st6[:, 2 * j + t2, :]
                                )
                        sl = slice(h * tph, (h + 1) * tph)
                        nc.vector.tensor_scalar(
                            veps[:, sl], mv[:, sl, 1], EPS, None, op0=OP.add
                        )
                        nc.scalar.activation(std[:, sl], veps[:, sl], ACT.Sqrt)
                        nc.vector.reciprocal(rstd[:, sl], std[:, sl])
                        nc.vector.scalar_tensor_tensor(
                            nmr[:, sl], mv[:, sl, 0], -1.0, rstd[:, sl],
                            op0=OP.mult, op1=OP.mult,
                        )
                    return raws, mv, rstd, nmr

                def apply_transpose(raws, ntiles, rstd, nmr, mv, xT, engine):
                    """xh = LN-apply (ScalarE Identity or DVE centering),
                    PE-transpose per 4 tiles, cast back to SBUF bf16."""
                    for j in range(max(ntiles // 4, 1)):
                        n4 = min(4, ntiles - 4 * j)
                        xh = xhpool.tile([128, 4, D], bf16, tag="xh")
                        for t4 in range(n4):
                            i = 4 * j + t4
                            jj, t2 = divmod(i, 2)
                            if engine == "scalar":
                                nc.scalar.activation(
                                    xh[:, t4, :], raws[jj][:, t2, :],
                                    ACT.Identity,
                                    bias=nmr[:, i : i + 1],
                                    scale=rstd[:, i : i + 1],
                                )
                            else:
                                # centering only; rstd folded downstream
                                nc.vector.tensor_scalar(
                                    xh[:, t4, :], raws[jj][:, t2, :],
                                    mv[:, i, 0:1], None, op0=OP.subtract,
                                )
                        ps = psT.tile([128, 4, 2, 128], bf16, tag="tr")
                        for t4 in range(n4):
                            i = 4 * j + t4
                            for dc in range(2):
                                nc.tensor.transpose(
                                    ps[:, t4, dc, :],
                                    xh[:, t4, 128 * dc : 128 * (dc + 1)],
                                    identb_sb,
                                )
                        src = ps[:, 0:n4, :, :].rearrange("p t d c -> p d t c")
                        dst = xT[:, :, 512 * j : 512 * j + 128 * n4].rearrange(
                            "p d (t c) -> p d t c", c=128
                        )
                        nc.vector.tensor_copy(dst, src)

                # ---- K: stats + apply(ScalarE) + projection ----
                rawsK, mvK, rstdK, nmrK = ln_stats(k_in.ap(), NTK, "k")
                xkT = stpool.tile([128, 2, TOKK], bf16, tag="xTkv")
                apply_transpose(rawsK, NTK, rstdK, nmrK, mvK, xkT, "scalar")
                for et in range(2):
                    for t in range(TOKK // 512):
                        ps = psA.tile([128, 512], f32, tag="projk")
                        for dc in range(2):
                            nc.tensor.matmul(
                                ps[:],
                                wk_s(dc, 128 * et, 128),
                                xkT[:, dc, 512 * t : 512 * (t + 1)],
                                start=(dc == 0),
                                stop=(dc == 1),
                            )
                        nc.vector.tensor_scalar(
                            kpT[:, et, 512 * t : 512 * (t + 1)],
                            ps[:], cf[:, CF_BIASK + et : CF_BIASK + et + 1],
                            None, op0=OP.add,
                        )
                # ---- Q: stats + apply(ScalarE) + projection ----
                rawsQ, mvQ, rstdQ, nmrQ = ln_stats(q_in.ap(), NTQ, "q")
                xqT = stpool.tile([128, 2, TOKQ], bf16, tag="xTq")
                apply_transpose(rawsQ, NTQ, rstdQ, nmrQ, mvQ, xqT, "scalar")
                for et in range(2):
                    ps = psA.tile([128, 512], f32, tag="projk")
                    for dc in range(2):
                        nc.tensor.matmul(
                            ps[:],
                            wq_s(dc, 128 * et, 128),
                            xqT[:, dc, 0:TOKQ],
                            start=(dc == 0),
                            stop=(dc == 1),
                        )
                    nc.vector.tensor_scalar(
                        qpT[:, et, :], ps[:],
                        cf[:, CF_BIASQ + et : CF_BIASQ + et + 1],
                        None, op0=OP.add,
                    )
                # ---- V: stats early (so its Sqrt precedes the Exp chain) ----
                rawsV, mvV, rstdV, nmrV = ln_stats(v_in.ap(), NTK, "v")

                # ============ Phase C part 1: scores + exp per (b, g) =======
                import os as _os
                _stage = _os.environ.get("KSTAGE", "full")
                PTs = {}
                for b in range(B):
                    for g in range(G):
                        bg = 8 * b + g
                        gp = 32 * (g % 4)
                        et = g // 4
                        PT = ptpool.tile([128, 16, NLOC], bf16, tag="PT")
                        PTs[bg] = PT
                        for quarter in range(4):
                            ps_s = psS.tile([128, 4, NLOC], f32, tag="scores")
                            for kc4 in range(4):
                                kc = 4 * quarter + kc4
                                nc.tensor.matmul(
                                    ps_s[:, kc4, :],
                                    kpT[gp : gp + 32, et,
                                        2048 * b + 128 * kc : 2048 * b + 128 * kc + 128],
                                    qpT[gp : gp + 32, et, NLOC * b : NLOC * b + NLOC],
                                    tile_position=(gp, 0),
                                )
                            nc.scalar.activation(
                                PT[:, 4 * quarter : 4 * quarter + 4, :],
                                ps_s[:, :, :],
                                ACT.Exp,
                                scale=SCALE,
                            )

                # ---- V: apply(DVE centering) + transpose + projection ------
                # (rstd applied per-token at the projection cast)
                xvT = stpool.tile([128, 2, TOKK], bf16, tag="xTkv")
                apply_transpose(rawsV, NTK, rstdV, nmrV, mvV, xvT, "vector")
                for t in range(NTK):
                    ps = psA.tile([128, D + 1], f32, tag="projk")
                    for dc in range(2):
                        nc.tensor.matmul(
                            ps[:],
                            xvT[:, dc, 128 * t : 128 * (t + 1)],
                            wv_s(dc),
                            start=(dc == 0),
                            stop=(dc == 1),
                        )
                    nc.vector.tensor_scalar(
                        vp[:, t, 0 : D + 1], ps[:], rstdV[:, t : t + 1], None,
                        op0=OP.mult,
                    )
                nc.vector.memset(vp[:, :, D + 1 : D + 2], 1.0)

                # ============ Phase C part 2: O = P @ [v|vw2|1] ============
                for b in range(B):
                    for g in range(G):
                        bg = 8 * b + g
                        PT = PTs[bg]
                        for qh in range(2):
                            ps_o = psA.tile([128, D + 2], f32, tag="projk")
                            for kc in range(16):
                                nc.tensor.matmul(
                                    ps_o[:],
                                    PT[:, kc, 128 * qh : 128 * qh + 128],
                                    vp[:, 16 * b + kc, :],
                                    start=(kc == 0),
                                    stop=(kc == 15),
                                )
                            nc.vector.tensor_copy(og[:, bg, qh, :], ps_o[:, 0:D])
                            nc.vector.tensor_copy(
                                ul[:, bg, qh, :], ps_o[:, D : D + 2]
                            )

                if _stage == "C":
                    dummy = spool.tile([128, D], f32, tag="dummy")
                    nc.vector.memset(dummy[:], 0.0)
                    for r in range(4):
                        nc.sync.dma_start(out_t[128 * r : 128 * r + 128, :], dummy[:])
                    stackA.close()
                    return
                stackA.close()

                # ============ Phase D: small stage + collective ============
                stackD = ExitStack()
                psB = stackD.enter_context(
                    tc.tile_pool(name="psB", bufs=2, space=MS.PSUM)
                )
                recl = spool.tile([128, 16, 2], f32, tag="recl")
                tt = spool.tile([128, 16, 2], f32, tag="tt")
                od = spool.tile([128, 16, 2], f32, tag="od")
                w1s = spool.tile([128, 16, 2], f32, tag="w1s")
                w2s = spool.tile([128, 16, 2], f32, tag="w2s")
                s_st = spool.tile([128, 2, 2], f32, tag="s_st")
                trow = spool.tile([1, 16], f32, tag="trow")
                trow8 = spool.tile([1, 128], f32, tag="trow8")
                t64 = spool.tile([1, 64], f32, tag="t64")
                t32 = spool.tile([1, 32], f32, tag="t32")

                nc.vector.reciprocal(recl[:], ul[:, :, :, 1])
                nc.vector.tensor_mul(tt[:], ul[:, :, :, 0], recl[:])
                ps_tp = psB.tile([16, 1], f32, tag="tpp")
                for qh in range(2):
                    nc.tensor.matmul(
                        ps_tp[:],
                        tt[:, :, qh],
                        onec_sb,
                        start=(qh == 0),
                        stop=(qh == 1),
                    )
                tp_sb = spool.tile([16, 1], f32, tag="tp_sb")
                nc.vector.tensor_copy(tp_sb[:], ps_tp[:])
                ag_in = dpool.tile([16, 1], f32, tag="ag_in")
                ag_out = dpool.tile([128, 1], f32, tag="ag_out")
                nc.sync.dma_start(ag_in[:], tp_sb[:])
                nc.gpsimd.collective_compute(
                    "AllGather",
                    OP.bypass,
                    ins=[ag_in.opt()],
                    outs=[ag_out.opt()],
                    replica_groups=[list(range(NCORES))],
                )
                nc.sync.dma_start(trow8[:], ag_out[:].rearrange("a b -> b a"))
                # Collective-independent DVE work first: w2s + the G2
                # recombination overlap the collective wait.
                nc.vector.tensor_scalar_mul(w2s[:], recl[:], 1.0 / G)
                for b in range(B):
                    for qh in range(2):
                        for g in range(G):
                            bg = 8 * b + g
                            if g == 0:
                                nc.vector.tensor_scalar_mul(
                                    G2[:, b, qh, :], og[:, bg, qh, :],
                                    w2s[:, bg, qh : qh + 1],
                                )
                            else:
                                nc.vector.affine_then_add(
                                    G2[:, b, qh, :], og[:, bg, qh, :], G2[:, b, qh, :],
                                    scale=w2s[:, bg, qh : qh + 1], bias=0.0,
                                )
                        nc.vector.tensor_add(
                            G2[:, b, qh, :], G2[:, b, qh, :], bv_sb
                        )
                nc.vector.tensor_add(t64[:], trow8[:, 0:64], trow8[:, 64:128])
                nc.vector.tensor_add(t32[:], t64[:, 0:32], t64[:, 32:64])
                nc.vector.tensor_add(trow[:], t32[:, 0:16], t32[:, 16:32])
                ps_tbc = psB.tile([128, 16], f32, tag="tbc")
                nc.tensor.matmul(ps_tbc[:], oner_sb[:], trow[:])
                for qh in range(2):
                    nc.vector.tensor_sub(od[:, :, qh], ps_tbc[:], tt[:, :, qh])
                nc.vector.tensor_scalar(
                    od[:], od[:], (N - 1) * b2s_eff, None, op0=OP.add
                )
                nc.vector.tensor_mul(w1s[:], od[:], recl[:])
                for b in range(B):
                    for qh in range(2):
                        nc.vector.reduce_sum(
                            s_st[:, qh, b : b + 1], od[:, 8 * b : 8 * b + 8, qh],
                            axis=AX,
                        )

                # ============ Phase E: G1 recombination (collective-gated) ==
                for b in range(B):
                    for qh in range(2):
                        for g in range(G):
                            bg = 8 * b + g
                            if g == 0:
                                nc.vector.tensor_scalar_mul(
                                    G1[:, b, qh, :], og[:, bg, qh, :],
                                    w1s[:, bg, qh : qh + 1],
                                )
                            else:
                                nc.vector.affine_then_add(
                                    G1[:, b, qh, :], og[:, bg, qh, :], G1[:, b, qh, :],
                                    scale=w1s[:, bg, qh : qh + 1], bias=0.0,
                                )

                # ============ Phase F: gate + output ============
                g1T = ppool.tile([128, 2, 4, 128], bf16, tag="g1T")
                for b in range(B):
                    for qh in range(2):
                        idx = 2 * b + qh
                        for dc in range(2):
                            ps = psB.tile([128, 128], f32, tag="g1tp")
                            nc.tensor.matmul(
                                ps[:],
                                G1[:, b, qh, 128 * dc : 128 * dc + 128],
                                identf_sb,
                                is_transpose=True,
                            )
                            nc.vector.tensor_copy(g1T[:, dc, idx, :], ps[:])
                for b in range(B):
                    for qh in range(2):
                        idx = 2 * b + qh
                        ps_a = psB.tile([128, D], f32, tag="A0")
                        for dc in range(2):
                            nc.tensor.matmul(
                                ps_a[:],
                                g1T[:, dc, idx, :],
                                w1_s(dc),
                                start=(dc == 0),
                                stop=(dc == 1),
                            )
                        gi = fpool.tile([128, D], f32, tag="gi")
                        nc.vector.affine_then_add(
                            gi[:], svec_sb, ps_a[:],
                            scale=s_st[:, qh, b : b + 1], bias=0.0,
                        )
                        nc.vector.tensor_add(gi[:], gi[:], mlb_sb)
                        gate = fpool.tile([128, D], f32, tag="gate")
                        nc.scalar.activation(gate[:], gi[:], ACT.Sigmoid)
                        of = fpool.tile([128, D], f32, tag="of")
                        nc.vector.tensor_mul(of[:], G2[:, b, qh, :], gate[:])
                        nc.sync.dma_start(
                            out_t[NLOC * b + 128 * qh : NLOC * b + 128 * qh + 128, :],
                            of[:],
                        )
                stackD.close()

            for _rep in range(repeat):
                _pipeline()

    return nc


def prepare(**inputs):
    q = np.asarray(inputs["q"], np.float32)
    k = np.asarray(inputs["k"], np.float32)
    v = np.asarray(inputs["v"], np.float32)
    ln_q_w = np.asarray(inputs["ln_q_w"], np.float64)
    ln_q_b = np.asarray(inputs["ln_q_b"], np.float64)
    ln_k_w = np.asarray(inputs["ln_k_w"], np.float64)
    ln_k_b = np.asarray(inputs["ln_k_b"], np.float64)
    ln_v_w = np.asarray(inputs["ln_v_w"], np.float64)
    ln_v_b = np.asarray(inputs["ln_v_b"], np.float64)
    Wq = np.asarray(inputs["Wq"], np.float64)
    Wk = np.asarray(inputs["Wk"], np.float64)
    Wv = np.asarray(inputs["Wv"], np.float64)
    W1 = np.asarray(inputs["W1"], np.float64)
    b1 = np.asarray(inputs["b1"], np.float64)
    W2 = np.asarray(inputs["W2"], np.float64)
    b2 = np.asarray(inputs["b2"], np.float64)
    mlp_w = np.asarray(inputs["mlp_w"], np.float64)
    mlp_b = np.asarray(inputs["mlp_b"], np.float64)

    # ---- host-side weight folding ----
    wqT = (Wq.T * ln_q_w[:, None]).astype(BF16)            # (d, e)
    wkT = (Wk.T * ln_k_w[:, None]).astype(BF16)
    wvT_eff = Wv.T * ln_v_w[:, None]                        # (d, e) float64
    biasq = (ln_q_b @ Wq.T).astype(np.float32)              # (e,)
    biask = (ln_k_b @ Wk.T).astype(np.float32)
    bias_v = ln_v_b @ Wv.T                                  # (e,) float64
    w2sum = W2.sum(axis=0)                                  # (d,)
    b2sum = float(b2.sum())
    w2e = wvT_eff @ w2sum                                   # (d,)
    wvT = np.concatenate([wvT_eff, w2e[:, None]], axis=1).astype(BF16)  # (d, 257)
    w1Tm_f = W1.T * (-CREL * mlp_w)[None, :]                # (d, e) float64
    w1Tm = w1Tm_f.astype(BF16)
    svec = (-CREL * mlp_w * b1 + bias_v @ w1Tm_f).astype(np.float32)
    b2s_eff = float(bias_v @ w2sum + b2sum)

    def fold128(m):
        """(256, X) -> (128, 2, X) -> (128, 2*X) blob block (p, (c x))."""
        x = np.asarray(m)
        cols = x.shape[1]
        return np.ascontiguousarray(
            x.reshape(2, 128, cols).transpose(1, 0, 2).reshape(128, 2 * cols)
        )

    cblob = np.concatenate(
        [
            np.eye(128, dtype=np.float64),
            fold128(wqT.astype(np.float64)),
            fold128(wkT.astype(np.float64)),
            fold128(wvT.astype(np.float64)),
            fold128(w1Tm.astype(np.float64)),
        ],
        axis=1,
    ).astype(BF16)
    assert cblob.shape == (128, CB_COLS)

    def bc(vec):
        return np.broadcast_to(
            np.asarray(vec, np.float32)[None, :], (128, D)
        ).copy()

    cfblob = np.concatenate(
        [
            np.eye(128, dtype=np.float32),
            bc(svec),
            bc(mlp_b),
            bc(bias_v),
            np.ascontiguousarray(biasq.reshape(2, 128).T),
            np.ascontiguousarray(biask.reshape(2, 128).T),
            np.ones((128, 1), np.float32),
        ],
        axis=1,
    ).astype(np.float32)
    assert cfblob.shape == (128, CF_COLS)

    # ---- b-major activations ----
    k_bm = np.ascontiguousarray(k.transpose(1, 0, 2).reshape(TOKK, D))
    v_bm = np.ascontiguousarray(v.transpose(1, 0, 2).reshape(TOKK, D))
    q_bm = q.transpose(1, 0, 2)                             # (B, N, D)

    common = {
        "k_in": k_bm,
        "v_in": v_bm,
        "cblob": cblob,
        "cfblob": cfblob,
        "ones_row": np.ones((1, 128), np.float32),
    }
    in_maps = []
    for i in range(NCORES):
        q_sl = np.ascontiguousarray(
            q_bm[:, i * NLOC : (i + 1) * NLOC, :].reshape(TOKQ, D)
        )
        in_maps.append({**common, "q_sl": q_sl})

    return in_maps, {"b2s_eff": b2s_eff}


def assemble(results):
    full = np.zeros((B, N, D), np.float32)
    for i in range(NCORES):
        o = np.asarray(results[i]["out"]).reshape(B, NLOC, D)
        full[:, i * NLOC : (i + 1) * NLOC, :] = o
    return np.ascontiguousarray(full.transpose(1, 0, 2))


def kernel(**inputs):
    from concourse import bass_utils

    in_maps, consts = prepare(**inputs)
    nc = _build(consts)
    nc.compile()
    res = bass_utils.run_bass_kernel_spmd(nc, in_maps, core_ids=list(range(NCORES)))
    global LAST_RESULT
    LAST_RESULT = res
    return assemble([res.results[i] for i in range(NCORES)])
